# revision 1
# baseline (speedup 1.0000x reference)
"""Multi-head causal attention (B=2, S=2048, E=1024, H=16, D=64) on 8 TRN2
NeuronCores. Sharding: batch (2) x head-groups (4 heads each) -> 8 cores.
Each core computes Q/K/V projections for its 4 heads, RoPE, flash-style
causal attention, and a partial output projection (its head columns of Wo);
the host sums the 4 partials per batch.

Layout notes:
- Q/K are produced directly in transposed [dims, S] layout by making the
  weight the stationary matmul operand. Per head, dims are permuted to
  [evens(32); odds(32)] so RoPE pair-halves are contiguous partition slices;
  the same permutation is applied to Q and K so scores are unchanged.
- Weight columns are arranged so psum chunk E holds the even-halves of all
  4 heads (rows 32h..32h+31 = head h) and chunk O the odd-halves. RoPE is
  then 6 full-width [128,512] DVE ops per S-chunk reading the two psums.
- Scores are computed transposed (scoresT[k,q]) so the AV matmul can use
  V as the stationary operand in natural [S, dims] layout; a ones column
  appended to V yields the softmax denominators in psum row 64.
- Softmax normalization: reciprocal (DVE) -> partition_broadcast (GPSIMD,
  attn ucode library) -> multiply during the psum->SBUF drain (DVE).
- All matmul inputs are bf16 (1 cycle/row on the PE) with fp32 psum
  accumulation; exp runs on the scalar engine reading two heads' scores
  from one 2-bank psum tile per op. Attention runs as two head-pair
  passes per q-chunk so the scores psum double-buffers (exp overlaps the
  next scores matmul). Measured ~260us/core on TRN2 silicon (reps=8
  in-NEFF repetition, marginal/8; ~112us PE-work floor for this dataflow).
"""

import sys

if "/opt/trn_rl_repo" not in sys.path:
    sys.path.insert(0, "/opt/trn_rl_repo")

import numpy as np
import ml_dtypes

B, S, E, H = 2, 2048, 1024, 16
D = E // H          # 64
HPC = 4             # heads per core
NCORES = 8
NE = E // 128       # 8 contraction chunks
NQ = S // 512       # 4 q-chunks
NK = S // 128       # 16 k-blocks
ROPE_BASE = 10000.0
ATTN_SCALE = 1.0 / np.sqrt(E)


def build_bass(reps=1):
    import concourse.bass as bass
    import concourse.mybir as mybir
    from concourse import bacc
    from concourse import library_config
    from concourse.tile import TileContext

    F32 = mybir.dt.float32
    BF16 = mybir.dt.bfloat16
    F32R = mybir.dt.float32r
    Exp = mybir.ActivationFunctionType.Exp

    nc = bacc.Bacc()

    xT_e = nc.declare_dram_parameter("xT", [E, S], BF16, isOutput=False)
    wq_e = nc.declare_dram_parameter("wq", [E, 2, 128], BF16, isOutput=False)
    wk_e = nc.declare_dram_parameter("wk", [E, 2, 128], BF16, isOutput=False)
    wv_e = nc.declare_dram_parameter("wv", [E, 256], BF16, isOutput=False)
    wo_e = nc.declare_dram_parameter("wo", [256, E], BF16, isOutput=False)
    cs_e = nc.declare_dram_parameter("cs", [128, S], BF16, isOutput=False)
    sn_e = nc.declare_dram_parameter("sn", [128, S], BF16, isOutput=False)
    dm_e = nc.declare_dram_parameter("dmask", [4, 128, 512], BF16, isOutput=False)
    out_e = nc.declare_dram_parameter("out", [S, E], F32, isOutput=True)

    nc.gpsimd.load_library(library_config.attn)

    with TileContext(nc) as tc:
        with (
            tc.tile_pool(name="wpool", bufs=1) as wpool,
            tc.tile_pool(name="xpool", bufs=1) as xpool,
            tc.tile_pool(name="qk", bufs=1) as qkpool,
            tc.tile_pool(name="vpool", bufs=1) as vpool,
            tc.tile_pool(name="epool", bufs=8) as epool,
            tc.tile_pool(name="rtmp", bufs=4) as rtmp,
            tc.tile_pool(name="atp", bufs=1) as atpool,
            tc.tile_pool(name="ypool", bufs=4) as ypool,
            tc.tile_pool(name="npool", bufs=4) as npool,
            tc.tile_pool(name="psA", bufs=2, space="PSUM") as psA,
            tc.tile_pool(name="psS", bufs=2, space="PSUM") as psS,
            tc.tile_pool(name="psO", bufs=1, space="PSUM") as psO,
        ):
          for _rep in range(reps):
            # ---- static inputs (x first: it gates the projection matmuls;
            # split per (e-chunk, 512-col slice) and loaded j-major so the
            # first projection group starts after ~1MB instead of 4MB) ----
            x_t = [[None] * NQ for _ in range(NE)]
            for j in range(NQ):
                for e in range(NE):
                    xt = xpool.tile([128, 512], BF16, tag=f"x{e}_{j}",
                                    name=f"x{e}_{j}")
                    nc.sync.dma_start(
                        xt[:], xT_e[128 * e : 128 * (e + 1),
                                    512 * j : 512 * (j + 1)])
                    x_t[e][j] = xt

            wq_t, wk_t, wv_t = [], [], []
            for e in range(NE):
                wqt = wpool.tile([128, 2, 128], BF16, tag=f"wq{e}", name=f"wq{e}")
                nc.sync.dma_start(
                    wqt[:], wq_e[128 * e : 128 * (e + 1)])
                wq_t.append(wqt)
                wkt = wpool.tile([128, 2, 128], BF16, tag=f"wk{e}", name=f"wk{e}")
                nc.sync.dma_start(
                    wkt[:], wk_e[128 * e : 128 * (e + 1)])
                wk_t.append(wkt)
                wvt = wpool.tile([128, 256], BF16, tag=f"wv{e}", name=f"wv{e}")
                nc.sync.dma_start(
                    wvt[:], wv_e[128 * e : 128 * (e + 1)])
                wv_t.append(wvt)
            cs_sb = wpool.tile([128, S], BF16, tag="cs")
            nc.sync.dma_start(cs_sb[:], cs_e[:])
            sn_sb = wpool.tile([128, S], BF16, tag="sn")
            nc.sync.dma_start(sn_sb[:], sn_e[:])
            dm_sb = wpool.tile([128, 4, 512], BF16, tag="dm")
            nc.sync.dma_start(dm_sb[:], dm_e.rearrange("r p c -> p r c"))
            wo_sb = wpool.tile([128, 2, E], BF16, tag="wo")
            nc.sync.dma_start(wo_sb[:], wo_e.rearrange("(c p) e -> p c e", p=128))

            # ---- projections + RoPE -------------------------------------------
            # qe_t[j]: [128, 512] bf16, rows 32h..32h+31 = head h even dims
            qe_t, qo_t, ke_t, ko_t = ([None] * NQ for _ in range(4))
            v_t = [None] * NK

            def emit_qk(w_t, j, nm, et_list, ot_list):
                sl = slice(512 * j, 512 * (j + 1))
                pe_ps = psA.tile([128, 512], F32, tag="pp", name=f"pe_{nm}{j}")
                po_ps = psA.tile([128, 512], F32, tag="pp", name=f"po_{nm}{j}")
                for e in range(NE):
                    nc.tensor.matmul(
                        pe_ps[:], w_t[e][:, 0, :], x_t[e][j][:],
                        start=(e == 0), stop=(e == NE - 1))
                for e in range(NE):
                    nc.tensor.matmul(
                        po_ps[:], w_t[e][:, 1, :], x_t[e][j][:],
                        start=(e == 0), stop=(e == NE - 1))
                # drain psums to bf16 once, then RoPE in 4x bf16 DVE mode
                pe_sb = rtmp.tile([128, 512], BF16, tag="pe_sb", name="pe_sb")
                po_sb = rtmp.tile([128, 512], BF16, tag="po_sb", name="po_sb")
                nc.vector.tensor_copy(pe_sb[:], pe_ps[:])
                nc.vector.tensor_copy(po_sb[:], po_ps[:])
                t1 = rtmp.tile([128, 512], BF16, tag="t1", name="t1")
                t2 = rtmp.tile([128, 512], BF16, tag="t2", name="t2")
                t3 = rtmp.tile([128, 512], BF16, tag="t3", name="t3")
                t4 = rtmp.tile([128, 512], BF16, tag="t4", name="t4")
                nc.vector.tensor_mul(t1[:], pe_sb[:], cs_sb[:, sl])
                nc.vector.tensor_mul(t2[:], po_sb[:], sn_sb[:, sl])
                nc.vector.tensor_mul(t3[:], pe_sb[:], sn_sb[:, sl])
                nc.vector.tensor_mul(t4[:], po_sb[:], cs_sb[:, sl])
                et = qkpool.tile([128, 512], BF16, tag=f"{nm}e{j}",
                                 name=f"{nm}e{j}")
                ot = qkpool.tile([128, 512], BF16, tag=f"{nm}o{j}",
                                 name=f"{nm}o{j}")
                nc.vector.tensor_sub(et[:], t1[:], t2[:])
                nc.vector.tensor_add(ot[:], t3[:], t4[:])
                et_list[j] = et
                ot_list[j] = ot

            def emit_v(i):
                # V: natural [S, dims] layout, ones column per head (65 wide)
                pv = psA.tile([128, 256], F32, tag="pp", name=f"pv{i}")
                for e in range(NE):
                    nc.tensor.matmul(
                        pv[:],
                        x_t[e][i // 4][:, 128 * (i % 4) : 128 * (i % 4) + 128],
                        wv_t[e][:],
                        start=(e == 0), stop=(e == NE - 1))
                vt = vpool.tile([128, 4, 65], BF16, tag=f"v{i}", name=f"v{i}")
                nc.vector.tensor_copy(
                    vt[:, :, 0:64], pv[:].rearrange("p (h d) -> p h d", d=64))
                nc.vector.memset(vt[:, :, 64], 1.0)
                v_t[i] = vt

            # Emission order tracks the attention dependency front: the
            # jq=3 i-loop consumes (v_t[i], ke[i//4], qe[3]) from i=0, so
            # V/K slice 0 and Q slice 3 come first; later Q slices are only
            # needed when their (later) jq pass starts.
            for step in range(NQ):
                for i in range(4 * step, 4 * step + 4):
                    emit_v(i)
                emit_qk(wk_t, step, "k", ke_t, ko_t)
                emit_qk(wq_t, NQ - 1 - step, "q", qe_t, qo_t)

            # ---- attention -----------------------------------------------------
            # jq descending: the longest i-loop (jq=3) starts first, the
            # shortest (jq=0) forms the kernel tail. Each jq runs as two
            # head-pair passes so the scores psum pool can double-buffer
            # (bufs=2): exp(i) overlaps scores(i+1) instead of serializing
            # the whole scores->exp->AV chain through one slot.
            for jq in range(NQ - 1, -1, -1):
                nblk = 4 * jq + 4
                at_c = [atpool.tile([128, 512], BF16, tag=f"at{c}_{jq}",
                                    name=f"at{c}_{jq}") for c in range(2)]
                for hp in range(2):
                    po = [psO.tile([65, 512], F32, tag=f"o{g}",
                                   name=f"po{hp}_{g}") for g in range(2)]
                    for i in range(nblk):
                        r = i - 4 * jq
                        q0 = 128 * max(r, 0)
                        w = 512 - q0
                        jsl = slice(128 * (i % 4), 128 * (i % 4) + 128)
                        ss = psS.tile([128, 2, 512], F32, tag="ss", name="ss")
                        for g in range(2):
                            h = 2 * hp + g
                            hr = slice(32 * h, 32 * h + 32)
                            nc.tensor.matmul(
                                ss[:, g, q0:512], ke_t[i // 4][hr, jsl],
                                qe_t[jq][hr, q0:512],
                                start=True, stop=False,
                                tile_position=(32 * h, 0))
                            nc.tensor.matmul(
                                ss[:, g, q0:512], ko_t[i // 4][hr, jsl],
                                qo_t[jq][hr, q0:512],
                                start=False, stop=True,
                                tile_position=(32 * h, 0))
                        et = epool.tile([128, 2, 512], BF16, tag="e")
                        nc.scalar.activation(
                            et[:, :, q0:512], ss[:, :, q0:512], Exp,
                            scale=ATTN_SCALE)
                        if r >= 0:
                            nc.vector.tensor_mul(
                                et[:, :, q0:512], et[:, :, q0:512],
                                dm_sb[:, r, None, q0:512].to_broadcast(
                                    (128, 2, w)))
                        for g in range(2):
                            h = 2 * hp + g
                            nc.tensor.matmul(
                                po[g][:, q0:512], v_t[i][:, h, :],
                                et[:, g, q0:512],
                                start=(i == 0), stop=(i == nblk - 1))

                    # normalize: at = po[0:64] * (1 / po[64]) -> bf16
                    for g in range(2):
                        rt = npool.tile([1, 512], BF16, tag="rt")
                        with nc.allow_low_precision(
                                reason="softmax denom recip in bf16"):
                            nc.vector.reciprocal(rt[:], po[g][64:65, :])
                        bt = npool.tile([64, 512], BF16, tag="bt")
                        nc.gpsimd.partition_broadcast(bt[:], rt[:])
                        nc.vector.tensor_mul(
                            at_c[hp][64 * g : 64 * g + 64, :],
                            po[g][0:64, :], bt[:])

                # output projection for this q range
                for qb in range(4):
                    lsl = slice(128 * qb, 128 * qb + 128)
                    orow = 128 * (4 * jq + qb)
                    for ec in range(2):
                        esl = slice(512 * ec, 512 * (ec + 1))
                        yp = psA.tile([128, 512], F32, tag="pp")
                        for c in range(2):
                            nc.tensor.matmul(
                                yp[:], at_c[c][:, lsl], wo_sb[:, c, esl],
                                start=(c == 0), stop=(c == 1))
                        ys = ypool.tile([128, 512], F32, tag="y")
                        nc.vector.tensor_copy(ys[:], yp[:])
                        nc.sync.dma_start(
                            out_e[orow : orow + 128, esl], ys[:])
    nc.finalize()
    return nc


def host_inputs(x, Wq, Wk, Wv, Wo):
    """Build the 8 per-core input maps (numpy, host-side shard/permute)."""
    perm = np.concatenate([np.arange(0, D, 2), np.arange(1, D, 2)])  # evens;odds
    d2 = D // 2
    theta = 1.0 / (ROPE_BASE ** (np.arange(d2, dtype=np.float64) * 2.0 / D))
    pos = np.arange(S, dtype=np.float64)
    ang = pos[None, :] * theta[:, None]              # [32, S]
    cs = np.tile(np.cos(ang), (4, 1)).astype(ml_dtypes.bfloat16)  # [128, S]
    sn = np.tile(np.sin(ang), (4, 1)).astype(ml_dtypes.bfloat16)

    dm = np.zeros((4, 128, 512), dtype=np.float32)
    k_idx = np.arange(128)[:, None]
    c_idx = np.arange(512)[None, :]
    for r in range(4):
        dm[r] = (k_idx <= c_idx - 128 * r).astype(np.float32)
    dm = dm.astype(ml_dtypes.bfloat16)

    in_maps = []
    for c in range(NCORES):
        b, g = divmod(c, HPC)
        heads = [HPC * g + t for t in range(HPC)]
        # evens chunk cols: head-major, 32 even dims each; odds chunk likewise
        ecols = np.concatenate([D * h + perm[:d2] for h in heads])
        ocols = np.concatenate([D * h + perm[d2:] for h in heads])
        vcols = np.concatenate([D * h + np.arange(D) for h in heads])
        wq = np.stack([Wq.T[:, ecols], Wq.T[:, ocols]], axis=1)  # [E, 2, 128]
        wk = np.stack([Wk.T[:, ecols], Wk.T[:, ocols]], axis=1)
        wv = Wv.T[:, vcols]                                      # [E, 256]
        wo = Wo[:, vcols].T.astype(ml_dtypes.bfloat16)           # [256, E]
        in_maps.append({
            "xT": np.ascontiguousarray(x[b].T).astype(ml_dtypes.bfloat16),
            "wq": np.ascontiguousarray(wq).astype(ml_dtypes.bfloat16),
            "wk": np.ascontiguousarray(wk).astype(ml_dtypes.bfloat16),
            "wv": np.ascontiguousarray(wv).astype(ml_dtypes.bfloat16),
            "wo": np.ascontiguousarray(wo),
            "cs": cs, "sn": sn, "dmask": dm,
        })
    return in_maps


_CACHED = {}


def kernel(x, Wq, Wk, Wv, Wo):
    from concourse.bass_utils import run_bass_kernel_spmd

    if "nc" not in _CACHED:
        _CACHED["nc"] = build_bass()
    nc = _CACHED["nc"]
    in_maps = host_inputs(
        np.asarray(x, dtype=np.float32), np.asarray(Wq, dtype=np.float32),
        np.asarray(Wk, dtype=np.float32), np.asarray(Wv, dtype=np.float32),
        np.asarray(Wo, dtype=np.float32))
    res = run_bass_kernel_spmd(nc, in_maps, core_ids=list(range(NCORES)))
    y = np.empty((B, S, E), dtype=np.float32)
    for b in range(B):
        y[b] = sum(res.results[HPC * b + g]["out"] for g in range(HPC))
    return y

